# revision 1
# baseline (speedup 1.0000x reference)
"""RNN-T JointNet kernel for 8 Trainium2 NeuronCores.

out[b,t,u,:] = gelu_tanh(enc[b,t]@We + dec[b,u]@Wd + b1) @ Wfc

Sharding: flatten (B=4, T=512) -> 2048 rows, 256 contiguous rows per core.
Core c handles batch b=c//2, time slice t0=(c%2)*256 .. +256. Each core only
needs its own enc slice and one batch's dec.

Per-core layout (all fp32):
  - encT (D=256, TC=256), decT (D=256, U=128): host-transposed so the first
    matmuls produce pe/pd with H on partitions, t/u on the free dim.
  - peb[h, t] = enc@We + b1 (4 h-tiles of (128, 256) in SBUF)
  - pd[h, u]  = dec@Wd      (4 h-tiles of (128, 128) in SBUF)
  - main loop over groups of TB=8 t's:
      DVE:  tmp[h, tb, u] = pd[h, u] + peb[h, t0+tb]   (broadcast APs)
      ACT:  hact = gelu_tanh(tmp)
      PE :  out_psum(u=128, v=512) = sum_ht hact[ht][:, tb]ᵀ-block @ Wfc[ht]
            (hact tile is the stationary operand, Wfc streams, N=512)
      DMA:  out_psum -> out[t] (contiguous 256 KiB)
"""

import sys

import numpy as np

sys.path.insert(0, "/opt/trn_rl_repo")

import concourse.bacc as bacc
import concourse.bass as bass
import concourse.mybir as mybir
import concourse.tile as tile
from concourse.bass_utils import run_bass_kernel_spmd

B, T, U, D, H, V = 4, 512, 128, 256, 512, 512
NCORES = 8
TC = (B * T) // NCORES  # 256 t-rows per core
TB = 8  # t's per main-loop group

_PROGRAM = None
LAST_RESULT = None


def _build():
    global _PROGRAM
    if _PROGRAM is not None:
        return _PROGRAM

    f32 = mybir.dt.float32
    # Bacc (not raw Bass): its compile() pipeline moves matmul waits onto
    # ldweights and splits >1-wait instructions via event semaphores —
    # walrus rejects matmuls carrying 2 sync waits otherwise.
    nc = bacc.Bacc("TRN2", target_bir_lowering=False)

    encT_d = nc.declare_dram_parameter("encT", (D, TC), f32, isOutput=False)
    decT_d = nc.declare_dram_parameter("decT", (D, U), f32, isOutput=False)
    w1_d = nc.declare_dram_parameter("W1", (2 * D, H), f32, isOutput=False)
    b1_d = nc.declare_dram_parameter("b1", (H, 1), f32, isOutput=False)
    wfc_d = nc.declare_dram_parameter("Wfc", (H, V), f32, isOutput=False)
    out_d = nc.declare_dram_parameter("out", (TC, U, V), f32, isOutput=True)

    GELU = mybir.ActivationFunctionType.Gelu_apprx_tanh
    IDENT = mybir.ActivationFunctionType.Identity

    with tile.TileContext(nc) as tc:
        with (
            tc.tile_pool(name="const", bufs=1) as cpool,
            tc.tile_pool(name="work", bufs=2) as wpool,
            tc.tile_pool(name="outsb", bufs=6) as osb_pool,
            tc.tile_pool(name="pro_ps", bufs=2, space="PSUM") as pro_ps,
            tc.tile_pool(name="out_ps", bufs=4, space="PSUM") as out_ps_pool,
        ):
            # W1 row-block i (128 rows of the 512-row input dim) lives at
            # cols [i*H, (i+1)*H). Blocks 0,1 = We; blocks 2,3 = Wd.
            w1_sb = cpool.tile([128, 4 * H], f32)
            wfc_sb = cpool.tile([128, 4 * V], f32)  # block ht = Wfc[ht*128:...]
            b1_sb = cpool.tile([128, 4], f32)  # col ht = b1[ht*128:(ht+1)*128]
            encT_sb = cpool.tile([128, 2 * TC], f32)
            decT_sb = cpool.tile([128, 2 * U], f32)
            peb_sb = cpool.tile([128, 4 * TC], f32)
            pd_sb = cpool.tile([128, 4 * U], f32)

            # One DMA per SBUF tile (3D APs): keeps the per-instruction sync
            # wait count low (walrus rejects >N waits on a matmul) and the
            # transfers large.
            nc.sync.dma_start(
                w1_sb.rearrange("p (i h) -> p i h", i=4),
                w1_d[:, :].rearrange("(i p) h -> p i h", p=128),
            )
            nc.sync.dma_start(
                wfc_sb.rearrange("p (i v) -> p i v", i=4),
                wfc_d[:, :].rearrange("(i p) v -> p i v", p=128),
            )
            nc.sync.dma_start(
                b1_sb, b1_d[:, :].rearrange("(i p) o -> p (i o)", p=128)
            )
            nc.sync.dma_start(
                encT_sb.rearrange("p (i t) -> p i t", i=2),
                encT_d[:, :].rearrange("(i p) t -> p i t", p=128),
            )
            nc.sync.dma_start(
                decT_sb.rearrange("p (i u) -> p i u", i=2),
                decT_d[:, :].rearrange("(i p) u -> p i u", p=128),
            )

            # Prologue: pe[h,t] = enc@We ; pdb[h,u] = dec@Wd + b1
            for ht in range(4):
                pe_ps = pro_ps.tile([128, TC], f32)
                for di in range(2):
                    nc.tensor.matmul(
                        pe_ps,
                        w1_sb[:, di * H + ht * 128 : di * H + (ht + 1) * 128],
                        encT_sb[:, di * TC : (di + 1) * TC],
                        start=(di == 0),
                        stop=(di == 1),
                    )
                nc.scalar.copy(peb_sb[:, ht * TC : (ht + 1) * TC], pe_ps)
                pd_ps = pro_ps.tile([128, U], f32)
                for di in range(2):
                    nc.tensor.matmul(
                        pd_ps,
                        w1_sb[:, (2 + di) * H + ht * 128 : (2 + di) * H + (ht + 1) * 128],
                        decT_sb[:, di * U : (di + 1) * U],
                        start=(di == 0),
                        stop=(di == 1),
                    )
                nc.scalar.activation(
                    pd_sb[:, ht * U : (ht + 1) * U],
                    pd_ps,
                    IDENT,
                    bias=b1_sb[:, ht : ht + 1],
                )

            # Main loop over u: ACT fuses the pd[:,u] add into the GELU as a
            # per-partition bias; h_u (h on partitions, t free) feeds the PE
            # as the stationary operand; DVE bounces PSUM->SBUF; DMA stores
            # (128 t, 512 v) slabs at fixed u.
            for u in range(U):
                hts = []
                for ht in range(4):
                    hact = wpool.tile([128, TC], f32, tag=f"h{ht}")
                    nc.scalar.activation(
                        hact,
                        peb_sb[:, ht * TC : (ht + 1) * TC],
                        GELU,
                        bias=pd_sb[:, ht * U + u : ht * U + u + 1],
                    )
                    hts.append(hact)
                for ts in range(TC // 128):
                    ops = out_ps_pool.tile([128, V], f32)
                    for ht in range(4):
                        nc.tensor.matmul(
                            ops,
                            hts[ht][:, ts * 128 : (ts + 1) * 128],
                            wfc_sb[:, ht * V : (ht + 1) * V],
                            start=(ht == 0),
                            stop=(ht == 3),
                        )
                    osb = osb_pool.tile([128, V], f32)
                    nc.vector.tensor_copy(osb, ops)
                    nc.sync.dma_start(
                        out_d[ts * 128 : (ts + 1) * 128, u : u + 1, :],
                        osb[:, None, :],
                    )

    nc.compile()
    _PROGRAM = nc
    return nc


def kernel(enc, dec, W1, b1, Wfc):
    global LAST_RESULT
    nc = _build()
    enc = np.asarray(enc, dtype=np.float32)
    dec = np.asarray(dec, dtype=np.float32)
    W1 = np.ascontiguousarray(np.asarray(W1, dtype=np.float32))
    b1 = np.ascontiguousarray(np.asarray(b1, dtype=np.float32).reshape(H, 1))
    Wfc = np.ascontiguousarray(np.asarray(Wfc, dtype=np.float32))

    in_maps = []
    for c in range(NCORES):
        b, t0 = c // 2, (c % 2) * TC
        in_maps.append(
            {
                "encT": np.ascontiguousarray(enc[b, t0 : t0 + TC, :].T),
                "decT": np.ascontiguousarray(dec[b].T),
                "W1": W1,
                "b1": b1,
                "Wfc": Wfc,
            }
        )

    LAST_RESULT = run_bass_kernel_spmd(nc, in_maps, list(range(NCORES)))

    out = np.empty((B, T, U, V), np.float32)
    for c in range(NCORES):
        b, t0 = c // 2, (c % 2) * TC
        out[b, t0 : t0 + TC] = LAST_RESULT.results[c]["out"]
    return out



# revision 6
# speedup vs baseline: 3.4269x; 3.4269x over previous
"""RNN-T JointNet kernel for 8 Trainium2 NeuronCores.

out[b,t,u,:] = gelu_tanh(enc[b,t]@We + dec[b,u]@Wd + b1) @ Wfc

Sharding: flatten (B=4, T=512) -> 2048 rows, 256 contiguous rows per core.
Core c handles batch b=c//2, time slice t0=(c%2)*256 .. +256.

Mixed precision: the fc matmul dominates (32768x512x512 per core) and fp32
matmuls run at 1/4 PE rate, so hact and Wfc are bf16 (1 cycle/row). The
pre-activation math (projections, broadcast add, gelu input) stays fp32 —
the DVE broadcast add can't use the 2x 16-bit mode anyway (stride-0
operand), so fp32 there is free. Output is stored bf16 (halves the 512 MiB
HBM write) and upcast on host. Norm rel err ~3e-3, well under the 2e-2 gate.

Per-core engine budget @64 groups of 2 u's (GPSIMD cannot touch PSUM,
so it gets the SBUF-only add and DVE evacuates PSUM):
  GPSIMD: broadcast add  tmp[h,(2u,t)] = peb[h,t] + pdb[h,u] (~120 us)
  ACT   : gelu (128,2048) per group -> hact bf16             (~130 us)
  PE    : 16 matmuls/group, hact (128x128) stationary, Wfc
          streams 512 -> psum (128t, 2x512v)                 (~220 us)
  DVE   : psum (128,1024) fp32 -> osb bf16                   (~170 us)
  SP    : 2 output DMAs/group, 256 KiB each, 2 KiB/partition (~80 us)
"""

import sys

import numpy as np

sys.path.insert(0, "/opt/trn_rl_repo")

import ml_dtypes

import concourse.bacc as bacc
import concourse.bass as bass
import concourse.mybir as mybir
import concourse.tile as tile
from concourse.bass_utils import run_bass_kernel_spmd

B, T, U, D, H, V = 4, 512, 128, 256, 512, 512
NCORES = 8
TC = (B * T) // NCORES  # 256 t-rows per core
UB = 2  # u's per main-loop group
NG = U // UB

_PROGRAM = None
LAST_RESULT = None


def _build():
    global _PROGRAM
    if _PROGRAM is not None:
        return _PROGRAM

    f32 = mybir.dt.float32
    bf16 = mybir.dt.bfloat16
    # Bacc (not raw Bass): its compile() pipeline moves matmul waits onto
    # ldweights and splits >1-wait instructions via event semaphores —
    # walrus rejects matmuls carrying 2 sync waits otherwise.
    nc = bacc.Bacc("TRN2", target_bir_lowering=False)

    encT_d = nc.declare_dram_parameter("encT", (D, TC), f32, isOutput=False)
    decT_d = nc.declare_dram_parameter("decT", (D, U), f32, isOutput=False)
    w1_d = nc.declare_dram_parameter("W1", (2 * D, H), f32, isOutput=False)
    b1_d = nc.declare_dram_parameter("b1", (H, 1), f32, isOutput=False)
    wfc_d = nc.declare_dram_parameter("Wfc", (H, V), bf16, isOutput=False)
    out_d = nc.declare_dram_parameter("out", (TC, U, V), bf16, isOutput=True)

    GELU = mybir.ActivationFunctionType.Gelu_apprx_tanh
    IDENT = mybir.ActivationFunctionType.Identity

    with tile.TileContext(nc) as tc:
        with (
            tc.tile_pool(name="const", bufs=1) as cpool,
            tc.tile_pool(name="tmps", bufs=2) as tpool,
            tc.tile_pool(name="hacts", bufs=2) as hpool,
            tc.tile_pool(name="outsb", bufs=4) as osb_pool,
            tc.tile_pool(name="pro_ps", bufs=2, space="PSUM") as pro_ps,
            tc.tile_pool(name="out_ps", bufs=2, space="PSUM") as out_ps_pool,
        ):
            # W1 row-block i (128 rows of the 512-row input dim) lives at
            # cols [i*H, (i+1)*H). Blocks 0,1 = We; blocks 2,3 = Wd.
            w1_sb = cpool.tile([128, 4 * H], f32)
            wfc_sb = cpool.tile([128, 4 * V], bf16)  # block ht = Wfc[ht*128:...]
            b1_sb = cpool.tile([128, 4], f32)  # col ht = b1[ht*128:(ht+1)*128]
            encT_sb = cpool.tile([128, 2 * TC], f32)
            decT_sb = cpool.tile([128, 2 * U], f32)
            peb_sb = cpool.tile([128, 4 * TC], f32)  # [ht*TC+t] = enc@We
            pd_sb = cpool.tile([128, 4 * U], f32)  # [ht*U+u] = dec@Wd + b1

            nc.sync.dma_start(
                w1_sb.rearrange("p (i h) -> p i h", i=4),
                w1_d[:, :].rearrange("(i p) h -> p i h", p=128),
            )
            nc.sync.dma_start(
                wfc_sb.rearrange("p (i v) -> p i v", i=4),
                wfc_d[:, :].rearrange("(i p) v -> p i v", p=128),
            )
            nc.sync.dma_start(
                b1_sb, b1_d[:, :].rearrange("(i p) o -> p (i o)", p=128)
            )
            nc.sync.dma_start(
                encT_sb.rearrange("p (i t) -> p i t", i=2),
                encT_d[:, :].rearrange("(i p) t -> p i t", p=128),
            )
            nc.sync.dma_start(
                decT_sb.rearrange("p (i u) -> p i u", i=2),
                decT_d[:, :].rearrange("(i p) u -> p i u", p=128),
            )

            # Prologue: peb[h,t] = enc@We ; pd[h,u] = dec@Wd + b1
            for ht in range(4):
                pe_ps = pro_ps.tile([128, TC], f32)
                for di in range(2):
                    nc.tensor.matmul(
                        pe_ps,
                        w1_sb[:, di * H + ht * 128 : di * H + (ht + 1) * 128],
                        encT_sb[:, di * TC : (di + 1) * TC],
                        start=(di == 0),
                        stop=(di == 1),
                    )
                nc.scalar.copy(peb_sb[:, ht * TC : (ht + 1) * TC], pe_ps)
                pd_ps = pro_ps.tile([128, U], f32)
                for di in range(2):
                    nc.tensor.matmul(
                        pd_ps,
                        w1_sb[:, (2 + di) * H + ht * 128 : (2 + di) * H + (ht + 1) * 128],
                        decT_sb[:, di * U : (di + 1) * U],
                        start=(di == 0),
                        stop=(di == 1),
                    )
                nc.scalar.activation(
                    pd_sb[:, ht * U : (ht + 1) * U],
                    pd_ps,
                    IDENT,
                    bias=b1_sb[:, ht : ht + 1],
                )

            # Broadcast-add source APs, iteration order (u, ht, t):
            #   peb (128, UB, 4, TC): u-dim stride 0
            #   pd  (128, UB, 4, 1->TC): t-dim stride 0
            peb_bc = (
                peb_sb.rearrange("p (i t) -> p i t", i=4)
                .unsqueeze(1)
                .broadcast_to((128, UB, 4, TC))
            )
            pd_iu = pd_sb.rearrange("p (i u) -> p i u", i=4)

            # Main loop over groups of UB u's.
            for g in range(NG):
                u0 = g * UB
                tmp = tpool.tile([128, UB * 4 * TC], f32, tag="tmp")
                pd_bc = (
                    pd_iu[:, :, u0 : u0 + UB]
                    .transpose([0, 2, 1])
                    .unsqueeze(3)
                    .broadcast_to((128, UB, 4, TC))
                )
                nc.gpsimd.tensor_tensor(
                    tmp.rearrange("p (u i t) -> p u i t", u=UB, i=4),
                    peb_bc,
                    pd_bc,
                    mybir.AluOpType.add,
                )
                hact = hpool.tile([128, UB * 4 * TC], bf16, tag="hact")
                nc.scalar.activation(hact, tmp, GELU)

                # psum tile (128 t, 2 banks): [:, ui*512:+512] = out rows for
                # (t-block ts, u0+ui); contraction over 4 h-blocks.
                for ts in range(TC // 128):
                    ops = out_ps_pool.tile([128, UB * V], f32)
                    for ui in range(UB):
                        for ht in range(4):
                            nc.tensor.matmul(
                                ops[:, ui * V : (ui + 1) * V],
                                hact[
                                    :,
                                    ui * 4 * TC
                                    + ht * TC
                                    + ts * 128 : ui * 4 * TC
                                    + ht * TC
                                    + ts * 128
                                    + 128,
                                ],
                                wfc_sb[:, ht * V : (ht + 1) * V],
                                start=(ht == 0),
                                stop=(ht == 3),
                            )
                    osb = osb_pool.tile([128, UB * V], bf16)
                    nc.vector.tensor_copy(osb, ops)
                    nc.sync.dma_start(
                        out_d[ts * 128 : (ts + 1) * 128, u0 : u0 + UB, :],
                        osb.rearrange("p (u v) -> p u v", u=UB),
                    )

    nc.compile()
    _PROGRAM = nc
    return nc


def kernel(enc, dec, W1, b1, Wfc):
    global LAST_RESULT
    nc = _build()
    enc = np.asarray(enc, dtype=np.float32)
    dec = np.asarray(dec, dtype=np.float32)
    W1 = np.ascontiguousarray(np.asarray(W1, dtype=np.float32))
    b1 = np.ascontiguousarray(np.asarray(b1, dtype=np.float32).reshape(H, 1))
    Wfc_bf = np.ascontiguousarray(
        np.asarray(Wfc, dtype=np.float32).astype(ml_dtypes.bfloat16)
    )

    in_maps = []
    for c in range(NCORES):
        b, t0 = c // 2, (c % 2) * TC
        in_maps.append(
            {
                "encT": np.ascontiguousarray(enc[b, t0 : t0 + TC, :].T),
                "decT": np.ascontiguousarray(dec[b].T),
                "W1": W1,
                "b1": b1,
                "Wfc": Wfc_bf,
            }
        )

    LAST_RESULT = run_bass_kernel_spmd(nc, in_maps, list(range(NCORES)))

    out = np.empty((B, T, U, V), np.float32)
    for c in range(NCORES):
        b, t0 = c // 2, (c % 2) * TC
        out[b, t0 : t0 + TC] = LAST_RESULT.results[c]["out"].astype(np.float32)
    return out


# revision 7
# speedup vs baseline: 3.6380x; 1.0616x over previous
"""RNN-T JointNet kernel for 8 Trainium2 NeuronCores.

out[b,t,u,:] = gelu_tanh(enc[b,t]@We + dec[b,u]@Wd + b1) @ Wfc

Sharding: flatten (B=4, T=512) -> 2048 rows, 256 contiguous rows per core.
Core c handles batch b=c//2, time slice t0=(c%2)*256 .. +256.

Mixed precision: the fc matmul dominates (32768x512x512 per core) and fp32
matmuls run at 1/4 PE rate, so hact and Wfc are bf16 (1 cycle/row). The
prologue projections are also bf16 (small, and it cuts the slow-p-state
startup ramp); the broadcast add + gelu input stay fp32. Output is stored
bf16 (halves the 512 MiB HBM write) and upcast on host. Norm rel err
~3e-3, well under the 2e-2 gate.

Per-core engine budget @64 groups of 2 u's (PE is the floor at ~237 us;
GPSIMD cannot touch PSUM, so DVE evacuates PSUM):
  PE    : 16 matmuls/group, hact (128x128) stationary, Wfc
          streams 512 -> psum (128t, 2x512v)                 (~235 us)
  GPSIMD: broadcast add tmp[h,(2u,t)] = peb[h,t] + pdb[h,u]
          for h-blocks 1..3 only                             (~175 us)
  ACT   : bias-fused gelu for h-block 0 (2 instrs) + one big
          gelu over h-blocks 1..3 -> hact bf16               (~165 us)
  DVE   : psum (128,1024) fp32 -> osb bf16                   (~160 us)
  SP    : 2 output DMAs/group, 256 KiB each, 2 KiB/partition (~80 us)
"""

import sys

import numpy as np

sys.path.insert(0, "/opt/trn_rl_repo")

import ml_dtypes

import concourse.bacc as bacc
import concourse.bass as bass
import concourse.mybir as mybir
import concourse.tile as tile
from concourse.bass_utils import run_bass_kernel_spmd

B, T, U, D, H, V = 4, 512, 128, 256, 512, 512
NCORES = 8
TC = (B * T) // NCORES  # 256 t-rows per core
UB = 2  # u's per main-loop group
NG = U // UB

_PROGRAM = None
LAST_RESULT = None


def _build():
    global _PROGRAM
    if _PROGRAM is not None:
        return _PROGRAM

    f32 = mybir.dt.float32
    bf16 = mybir.dt.bfloat16
    # Bacc (not raw Bass): its compile() pipeline moves matmul waits onto
    # ldweights and splits >1-wait instructions via event semaphores —
    # walrus rejects matmuls carrying 2 sync waits otherwise.
    nc = bacc.Bacc("TRN2", target_bir_lowering=False)

    encT_d = nc.declare_dram_parameter("encT", (D, TC), bf16, isOutput=False)
    decT_d = nc.declare_dram_parameter("decT", (D, U), bf16, isOutput=False)
    w1_d = nc.declare_dram_parameter("W1", (2 * D, H), bf16, isOutput=False)
    b1_d = nc.declare_dram_parameter("b1", (H, 1), f32, isOutput=False)
    wfc_d = nc.declare_dram_parameter("Wfc", (H, V), bf16, isOutput=False)
    out_d = nc.declare_dram_parameter("out", (TC, U, V), bf16, isOutput=True)

    GELU = mybir.ActivationFunctionType.Gelu_apprx_tanh
    IDENT = mybir.ActivationFunctionType.Identity

    with tile.TileContext(nc) as tc:
        with (
            tc.tile_pool(name="const", bufs=1) as cpool,
            tc.tile_pool(name="tmps", bufs=2) as tpool,
            tc.tile_pool(name="hacts", bufs=2) as hpool,
            tc.tile_pool(name="outsb", bufs=4) as osb_pool,
            tc.tile_pool(name="pro_ps", bufs=2, space="PSUM") as pro_ps,
            tc.tile_pool(name="out_ps", bufs=2, space="PSUM") as out_ps_pool,
        ):
            # W1 row-block i (128 rows of the 512-row input dim) lives at
            # cols [i*H, (i+1)*H). Blocks 0,1 = We; blocks 2,3 = Wd.
            w1_sb = cpool.tile([128, 4 * H], bf16)
            wfc_sb = cpool.tile([128, 4 * V], bf16)  # block ht = Wfc[ht*128:...]
            b1_sb = cpool.tile([128, 4], f32)  # col ht = b1[ht*128:(ht+1)*128]
            encT_sb = cpool.tile([128, 2 * TC], bf16)
            decT_sb = cpool.tile([128, 2 * U], bf16)
            peb_sb = cpool.tile([128, 4 * TC], f32)  # [ht*TC+t] = enc@We
            pd_sb = cpool.tile([128, 4 * U], f32)  # [ht*U+u] = dec@Wd + b1

            nc.sync.dma_start(
                w1_sb.rearrange("p (i h) -> p i h", i=4),
                w1_d[:, :].rearrange("(i p) h -> p i h", p=128),
            )
            nc.sync.dma_start(
                wfc_sb.rearrange("p (i v) -> p i v", i=4),
                wfc_d[:, :].rearrange("(i p) v -> p i v", p=128),
            )
            nc.sync.dma_start(
                b1_sb, b1_d[:, :].rearrange("(i p) o -> p (i o)", p=128)
            )
            nc.sync.dma_start(
                encT_sb.rearrange("p (i t) -> p i t", i=2),
                encT_d[:, :].rearrange("(i p) t -> p i t", p=128),
            )
            nc.sync.dma_start(
                decT_sb.rearrange("p (i u) -> p i u", i=2),
                decT_d[:, :].rearrange("(i p) u -> p i u", p=128),
            )

            # Prologue: peb[h,t] = enc@We ; pd[h,u] = dec@Wd + b1
            for ht in range(4):
                pe_ps = pro_ps.tile([128, TC], f32)
                for di in range(2):
                    nc.tensor.matmul(
                        pe_ps,
                        w1_sb[:, di * H + ht * 128 : di * H + (ht + 1) * 128],
                        encT_sb[:, di * TC : (di + 1) * TC],
                        start=(di == 0),
                        stop=(di == 1),
                    )
                nc.scalar.copy(peb_sb[:, ht * TC : (ht + 1) * TC], pe_ps)
                pd_ps = pro_ps.tile([128, U], f32)
                for di in range(2):
                    nc.tensor.matmul(
                        pd_ps,
                        w1_sb[:, (2 + di) * H + ht * 128 : (2 + di) * H + (ht + 1) * 128],
                        decT_sb[:, di * U : (di + 1) * U],
                        start=(di == 0),
                        stop=(di == 1),
                    )
                nc.scalar.activation(
                    pd_sb[:, ht * U : (ht + 1) * U],
                    pd_ps,
                    IDENT,
                    bias=b1_sb[:, ht : ht + 1],
                )

            # Broadcast-add source APs for h-blocks 1..3, iteration order
            # (u, ht, t): peb u-dim stride 0; pd t-dim stride 0.
            peb_bc = (
                peb_sb[:, TC : 4 * TC]
                .rearrange("p (i t) -> p i t", i=3)
                .unsqueeze(1)
                .broadcast_to((128, UB, 3, TC))
            )
            pd_iu = pd_sb.rearrange("p (i u) -> p i u", i=4)

            # Main loop over groups of UB u's.
            for g in range(NG):
                u0 = g * UB
                hact = hpool.tile([128, UB * 4 * TC], bf16, tag="hact")
                # h-block 0: gelu straight from peb with pd as per-partition
                # bias — skips the explicit add for this block.
                for ui in range(UB):
                    nc.scalar.activation(
                        hact[:, ui * 4 * TC : ui * 4 * TC + TC],
                        peb_sb[:, 0:TC],
                        GELU,
                        bias=pd_sb[:, u0 + ui : u0 + ui + 1],
                    )
                # h-blocks 1..3: GPSIMD broadcast add, then one big gelu.
                tmp = tpool.tile([128, UB * 3 * TC], f32, tag="tmp")
                pd_bc = (
                    pd_iu[:, 1:4, u0 : u0 + UB]
                    .transpose([0, 2, 1])
                    .unsqueeze(3)
                    .broadcast_to((128, UB, 3, TC))
                )
                nc.gpsimd.tensor_tensor(
                    tmp.rearrange("p (u i t) -> p u i t", u=UB, i=3),
                    peb_bc,
                    pd_bc,
                    mybir.AluOpType.add,
                )
                nc.scalar.activation(
                    hact.rearrange("p (u x) -> p u x", u=UB)[:, :, TC : 4 * TC],
                    tmp.rearrange("p (u x) -> p u x", u=UB),
                    GELU,
                )

                # psum tile (128 t, 2 banks): [:, ui*512:+512] = out rows for
                # (t-block ts, u0+ui); contraction over 4 h-blocks.
                for ts in range(TC // 128):
                    ops = out_ps_pool.tile([128, UB * V], f32)
                    for ui in range(UB):
                        for ht in range(4):
                            nc.tensor.matmul(
                                ops[:, ui * V : (ui + 1) * V],
                                hact[
                                    :,
                                    ui * 4 * TC
                                    + ht * TC
                                    + ts * 128 : ui * 4 * TC
                                    + ht * TC
                                    + ts * 128
                                    + 128,
                                ],
                                wfc_sb[:, ht * V : (ht + 1) * V],
                                start=(ht == 0),
                                stop=(ht == 3),
                            )
                    osb = osb_pool.tile([128, UB * V], bf16)
                    nc.vector.tensor_copy(osb, ops)
                    nc.sync.dma_start(
                        out_d[ts * 128 : (ts + 1) * 128, u0 : u0 + UB, :],
                        osb.rearrange("p (u v) -> p u v", u=UB),
                    )

    nc.compile()
    _PROGRAM = nc
    return nc


def kernel(enc, dec, W1, b1, Wfc):
    global LAST_RESULT
    nc = _build()
    bf = ml_dtypes.bfloat16
    enc = np.asarray(enc, dtype=np.float32)
    dec = np.asarray(dec, dtype=np.float32)
    W1_bf = np.ascontiguousarray(np.asarray(W1, dtype=np.float32).astype(bf))
    b1 = np.ascontiguousarray(np.asarray(b1, dtype=np.float32).reshape(H, 1))
    Wfc_bf = np.ascontiguousarray(np.asarray(Wfc, dtype=np.float32).astype(bf))

    in_maps = []
    for c in range(NCORES):
        b, t0 = c // 2, (c % 2) * TC
        in_maps.append(
            {
                "encT": np.ascontiguousarray(enc[b, t0 : t0 + TC, :].T.astype(bf)),
                "decT": np.ascontiguousarray(dec[b].T.astype(bf)),
                "W1": W1_bf,
                "b1": b1,
                "Wfc": Wfc_bf,
            }
        )

    LAST_RESULT = run_bass_kernel_spmd(nc, in_maps, list(range(NCORES)))

    out = np.empty((B, T, U, V), np.float32)
    for c in range(NCORES):
        b, t0 = c // 2, (c % 2) * TC
        out[b, t0 : t0 + TC] = LAST_RESULT.results[c]["out"].astype(np.float32)
    return out


# revision 10
# speedup vs baseline: 3.7248x; 1.0238x over previous
"""RNN-T JointNet kernel for 8 Trainium2 NeuronCores.

out[b,t,u,:] = gelu_tanh(enc[b,t]@We + dec[b,u]@Wd + b1) @ Wfc

Sharding: flatten (B=4, T=512) -> 2048 rows, 256 contiguous rows per core.
Core c handles batch b=c//2, time slice t0=(c%2)*256 .. +256.

Mixed precision: the fc matmul dominates (32768x512x512 per core) and fp32
matmuls run at 1/4 PE rate, so hact and Wfc are bf16 (1 cycle/row). The
prologue projections are also bf16 (small, and it cuts the slow-p-state
startup ramp); the broadcast add + gelu input stay fp32. Output is stored
bf16 (halves the 512 MiB HBM write) and upcast on host. Norm rel err
~3e-3, well under the 2e-2 gate.

Per-core engine budget @64 groups of 2 u's (PE is the floor at ~237 us;
GPSIMD cannot touch PSUM, so DVE evacuates PSUM):
  PE    : 16 matmuls/group, hact (128x128) stationary, Wfc
          streams 512 -> psum (128t, 2x512v)                 (~235 us)
  GPSIMD: broadcast add tmp[h,(2u,t)] = peb[h,t] + pdb[h,u]
          for h-blocks 1..3 only                             (~175 us)
  ACT   : bias-fused gelu for h-block 0 (2 instrs) + one big
          gelu over h-blocks 1..3 -> hact bf16               (~165 us)
  DVE   : psum (128,1024) fp32 -> osb bf16                   (~160 us)
  SP    : 2 output DMAs/group, 256 KiB each, 2 KiB/partition (~80 us)
"""

import sys

import numpy as np

sys.path.insert(0, "/opt/trn_rl_repo")

import ml_dtypes

import concourse.bacc as bacc
import concourse.bass as bass
import concourse.mybir as mybir
import concourse.tile as tile
from concourse.bass_utils import run_bass_kernel_spmd

B, T, U, D, H, V = 4, 512, 128, 256, 512, 512
NCORES = 8
TC = (B * T) // NCORES  # 256 t-rows per core
UB = 2  # u's per main-loop group
NG = U // UB

_PROGRAM = None
LAST_RESULT = None


def _build():
    global _PROGRAM
    if _PROGRAM is not None:
        return _PROGRAM

    f32 = mybir.dt.float32
    bf16 = mybir.dt.bfloat16
    # Bacc (not raw Bass): its compile() pipeline moves matmul waits onto
    # ldweights and splits >1-wait instructions via event semaphores —
    # walrus rejects matmuls carrying 2 sync waits otherwise.
    nc = bacc.Bacc("TRN2", target_bir_lowering=False)

    encT_d = nc.declare_dram_parameter("encT", (D, TC), bf16, isOutput=False)
    decT_d = nc.declare_dram_parameter("decT", (D, U), bf16, isOutput=False)
    w1_d = nc.declare_dram_parameter("W1", (2 * D, H), bf16, isOutput=False)
    b1_d = nc.declare_dram_parameter("b1", (H, 1), f32, isOutput=False)
    wfc_d = nc.declare_dram_parameter("Wfc", (H, V), bf16, isOutput=False)
    out_d = nc.declare_dram_parameter("out", (TC, U, V), bf16, isOutput=True)

    GELU = mybir.ActivationFunctionType.Gelu_apprx_tanh
    IDENT = mybir.ActivationFunctionType.Identity

    with tile.TileContext(nc) as tc:
        with (
            tc.tile_pool(name="const", bufs=1) as cpool,
            tc.tile_pool(name="tmps", bufs=2) as tpool,
            tc.tile_pool(name="hacts", bufs=2) as hpool,
            tc.tile_pool(name="outsb", bufs=4) as osb_pool,
            tc.tile_pool(name="pro_ps", bufs=2, space="PSUM") as pro_ps,
            tc.tile_pool(name="out_ps", bufs=2, space="PSUM") as out_ps_pool,
        ):
            # W1 row-block i (128 rows of the 512-row input dim) lives at
            # cols [i*H, (i+1)*H). Blocks 0,1 = We; blocks 2,3 = Wd.
            w1_sb = cpool.tile([128, 4 * H], bf16)
            wfc_sb = cpool.tile([128, 4 * V], bf16)  # block ht = Wfc[ht*128:...]
            b1_sb = cpool.tile([128, 4], f32)  # col ht = b1[ht*128:(ht+1)*128]
            encT_sb = cpool.tile([128, 2 * TC], bf16)
            decT_sb = cpool.tile([128, 2 * U], bf16)
            peb_sb = cpool.tile([128, 4 * TC], f32)  # [ht*TC+t] = enc@We
            pd_sb = cpool.tile([128, 4 * U], f32)  # [ht*U+u] = dec@Wd + b1

            # Prologue inputs first; wfc last (first needed ~25us in).
            nc.sync.dma_start(
                w1_sb.rearrange("p (i h) -> p i h", i=4),
                w1_d[:, :].rearrange("(i p) h -> p i h", p=128),
            )
            nc.sync.dma_start(
                encT_sb.rearrange("p (i t) -> p i t", i=2),
                encT_d[:, :].rearrange("(i p) t -> p i t", p=128),
            )
            nc.sync.dma_start(
                decT_sb.rearrange("p (i u) -> p i u", i=2),
                decT_d[:, :].rearrange("(i p) u -> p i u", p=128),
            )
            nc.sync.dma_start(
                b1_sb, b1_d[:, :].rearrange("(i p) o -> p (i o)", p=128)
            )
            nc.sync.dma_start(
                wfc_sb.rearrange("p (i v) -> p i v", i=4),
                wfc_d[:, :].rearrange("(i p) v -> p i v", p=128),
            )

            # Prologue: peb[h,t] = enc@We ; pd[h,u] = dec@Wd + b1.
            # PSUM evacuation on DVE (idle early) so ACT can start the first
            # groups' bias-fused gelus as soon as each (peb, pd) block lands.
            for ht in range(4):
                pe_ps = pro_ps.tile([128, TC], f32)
                for di in range(2):
                    nc.tensor.matmul(
                        pe_ps,
                        w1_sb[:, di * H + ht * 128 : di * H + (ht + 1) * 128],
                        encT_sb[:, di * TC : (di + 1) * TC],
                        start=(di == 0),
                        stop=(di == 1),
                    )
                nc.vector.tensor_copy(peb_sb[:, ht * TC : (ht + 1) * TC], pe_ps)
                pd_ps = pro_ps.tile([128, U], f32)
                for di in range(2):
                    nc.tensor.matmul(
                        pd_ps,
                        w1_sb[:, (2 + di) * H + ht * 128 : (2 + di) * H + (ht + 1) * 128],
                        decT_sb[:, di * U : (di + 1) * U],
                        start=(di == 0),
                        stop=(di == 1),
                    )
                nc.vector.tensor_scalar_add(
                    pd_sb[:, ht * U : (ht + 1) * U],
                    pd_ps,
                    b1_sb[:, ht : ht + 1],
                )

            # Broadcast-add source APs for h-blocks 1..3, iteration order
            # (u, ht, t): peb u-dim stride 0; pd t-dim stride 0.
            peb_bc = (
                peb_sb[:, TC : 4 * TC]
                .rearrange("p (i t) -> p i t", i=3)
                .unsqueeze(1)
                .broadcast_to((128, UB, 3, TC))
            )
            pd_iu = pd_sb.rearrange("p (i u) -> p i u", i=4)

            # Main loop over groups of UB u's.
            for g in range(NG):
                u0 = g * UB
                hact = hpool.tile([128, UB * 4 * TC], bf16, tag="hact")
                if g < 2:
                    # First groups: all h-blocks via ACT bias-fused gelu —
                    # no GPSIMD add in the dependence chain, so PE's output
                    # matmuls start while the prologue is still draining.
                    for ui in range(UB):
                        for ht in range(4):
                            nc.scalar.activation(
                                hact[
                                    :, ui * 4 * TC + ht * TC : ui * 4 * TC + (ht + 1) * TC
                                ],
                                peb_sb[:, ht * TC : (ht + 1) * TC],
                                GELU,
                                bias=pd_sb[:, ht * U + u0 + ui : ht * U + u0 + ui + 1],
                            )
                else:
                    # h-block 0: gelu straight from peb with pd as
                    # per-partition bias — skips the explicit add.
                    for ui in range(UB):
                        nc.scalar.activation(
                            hact[:, ui * 4 * TC : ui * 4 * TC + TC],
                            peb_sb[:, 0:TC],
                            GELU,
                            bias=pd_sb[:, u0 + ui : u0 + ui + 1],
                        )
                    # h-blocks 1..3: GPSIMD broadcast add, then one big gelu.
                    tmp = tpool.tile([128, UB * 3 * TC], f32, tag="tmp")
                    pd_bc = (
                        pd_iu[:, 1:4, u0 : u0 + UB]
                        .transpose([0, 2, 1])
                        .unsqueeze(3)
                        .broadcast_to((128, UB, 3, TC))
                    )
                    nc.gpsimd.tensor_tensor(
                        tmp.rearrange("p (u i t) -> p u i t", u=UB, i=3),
                        peb_bc,
                        pd_bc,
                        mybir.AluOpType.add,
                    )
                    nc.scalar.activation(
                        hact.rearrange("p (u x) -> p u x", u=UB)[:, :, TC : 4 * TC],
                        tmp.rearrange("p (u x) -> p u x", u=UB),
                        GELU,
                    )

                # psum tile (128 t, 2 banks): [:, ui*512:+512] = out rows for
                # (t-block ts, u0+ui); contraction over 4 h-blocks.
                for ts in range(TC // 128):
                    ops = out_ps_pool.tile([128, UB * V], f32)
                    for ui in range(UB):
                        for ht in range(4):
                            nc.tensor.matmul(
                                ops[:, ui * V : (ui + 1) * V],
                                hact[
                                    :,
                                    ui * 4 * TC
                                    + ht * TC
                                    + ts * 128 : ui * 4 * TC
                                    + ht * TC
                                    + ts * 128
                                    + 128,
                                ],
                                wfc_sb[:, ht * V : (ht + 1) * V],
                                start=(ht == 0),
                                stop=(ht == 3),
                            )
                    osb = osb_pool.tile([128, UB * V], bf16)
                    nc.vector.tensor_copy(osb, ops)
                    nc.sync.dma_start(
                        out_d[ts * 128 : (ts + 1) * 128, u0 : u0 + UB, :],
                        osb.rearrange("p (u v) -> p u v", u=UB),
                    )

    nc.compile()
    _PROGRAM = nc
    return nc


def kernel(enc, dec, W1, b1, Wfc):
    global LAST_RESULT
    nc = _build()
    bf = ml_dtypes.bfloat16
    enc = np.asarray(enc, dtype=np.float32)
    dec = np.asarray(dec, dtype=np.float32)
    W1_bf = np.ascontiguousarray(np.asarray(W1, dtype=np.float32).astype(bf))
    b1 = np.ascontiguousarray(np.asarray(b1, dtype=np.float32).reshape(H, 1))
    Wfc_bf = np.ascontiguousarray(np.asarray(Wfc, dtype=np.float32).astype(bf))

    in_maps = []
    for c in range(NCORES):
        b, t0 = c // 2, (c % 2) * TC
        in_maps.append(
            {
                "encT": np.ascontiguousarray(enc[b, t0 : t0 + TC, :].T.astype(bf)),
                "decT": np.ascontiguousarray(dec[b].T.astype(bf)),
                "W1": W1_bf,
                "b1": b1,
                "Wfc": Wfc_bf,
            }
        )

    LAST_RESULT = run_bass_kernel_spmd(nc, in_maps, list(range(NCORES)))

    out = np.empty((B, T, U, V), np.float32)
    for c in range(NCORES):
        b, t0 = c // 2, (c % 2) * TC
        out[b, t0 : t0 + TC] = LAST_RESULT.results[c]["out"].astype(np.float32)
    return out


# revision 14
# speedup vs baseline: 3.7289x; 1.0011x over previous
"""RNN-T JointNet kernel for 8 Trainium2 NeuronCores.

out[b,t,u,:] = gelu_tanh(enc[b,t]@We + dec[b,u]@Wd + b1) @ Wfc

Sharding: flatten (B=4, T=512) -> 2048 rows, 256 contiguous rows per core.
Core c handles batch b=c//2, time slice t0=(c%2)*256 .. +256.

Mixed precision: the fc matmul dominates (32768x512x512 per core) and fp32
matmuls run at 1/4 PE rate, so hact and Wfc are bf16 (1 cycle/row). The
prologue projections are also bf16; the broadcast add + gelu input stay
fp32. Output is stored bf16 (halves the 512 MiB HBM write) and upcast on
host. Norm rel err ~3.8e-3, well under the 2e-2 gate.

All inputs are pre-tiled on host into the exact (128, free) SBUF layouts so
every input DMA is a contiguous >=1KiB-per-partition-line copy (the
transposed loads otherwise emit 512B descriptors and stretch startup).

Per-core engine budget @64 groups of 2 u's (PE is the floor at ~225 us;
GPSIMD cannot touch PSUM, so DVE evacuates PSUM):
  PE    : 16 matmuls/group, hact (128x128) stationary, Wfc
          streams 512 -> psum (128t, 2x512v)                 (~225 us)
  GPSIMD: broadcast add tmp[h,(2u,t)] = peb[h,t] + pdb[h,u]
          for h-blocks 1..3 only                             (~180 us)
  ACT   : bias-fused gelu for h-block 0 (2 instrs) + one big
          gelu over h-blocks 1..3 -> hact bf16               (~170 us)
  DVE   : psum (128,1024) fp32 -> osb bf16, prologue evac    (~170 us)
  SP    : 2 output DMAs/group, 256 KiB each, 2 KiB/partition (~80 us)
"""

import sys

import numpy as np

sys.path.insert(0, "/opt/trn_rl_repo")

import ml_dtypes

import concourse.bacc as bacc
import concourse.bass as bass
import concourse.mybir as mybir
import concourse.tile as tile
from concourse.bass_utils import run_bass_kernel_spmd

B, T, U, D, H, V = 4, 512, 128, 256, 512, 512
NCORES = 8
TC = (B * T) // NCORES  # 256 t-rows per core
UB = 2  # u's per main-loop group
NG = U // UB

_PROGRAM = None
LAST_RESULT = None


def _build():
    global _PROGRAM
    if _PROGRAM is not None:
        return _PROGRAM

    f32 = mybir.dt.float32
    bf16 = mybir.dt.bfloat16
    # Bacc (not raw Bass): its compile() pipeline moves matmul waits onto
    # ldweights and splits >1-wait instructions via event semaphores —
    # walrus rejects matmuls carrying 2 sync waits otherwise.
    nc = bacc.Bacc("TRN2", target_bir_lowering=False)

    # All inputs pre-tiled host-side to partition-major (128, free) layouts.
    w1we_d = nc.declare_dram_parameter("w1we", (128, 2 * H), bf16, isOutput=False)
    w1wd_d = nc.declare_dram_parameter("w1wd", (128, 2 * H), bf16, isOutput=False)
    encT_d = nc.declare_dram_parameter("encT", (128, 2 * TC), bf16, isOutput=False)
    decT_d = nc.declare_dram_parameter("decT", (128, 2 * U), bf16, isOutput=False)
    b1_d = nc.declare_dram_parameter("b1", (128, 4), f32, isOutput=False)
    wfc_d = nc.declare_dram_parameter("Wfc", (128, 4 * V), bf16, isOutput=False)
    out_d = nc.declare_dram_parameter("out", (TC, U, V), bf16, isOutput=True)

    GELU = mybir.ActivationFunctionType.Gelu_apprx_tanh

    with tile.TileContext(nc) as tc:
        with (
            tc.tile_pool(name="const", bufs=1) as cpool,
            tc.tile_pool(name="tmps", bufs=2) as tpool,
            tc.tile_pool(name="hacts", bufs=2) as hpool,
            tc.tile_pool(name="outsb", bufs=4) as osb_pool,
            tc.tile_pool(name="pro_ps", bufs=2, space="PSUM") as pro_ps,
            tc.tile_pool(name="out_ps", bufs=2, space="PSUM") as out_ps_pool,
        ):
            # w1we col-block di*H+h = We[di*128+p, h]; w1wd likewise for Wd.
            w1we_sb = cpool.tile([128, 2 * H], bf16)
            w1wd_sb = cpool.tile([128, 2 * H], bf16)
            wfc_sb = cpool.tile([128, 4 * V], bf16)  # block ht = Wfc[ht*128:...]
            b1_sb = cpool.tile([128, 4], f32)  # col ht = b1[ht*128:(ht+1)*128]
            encT_sb = cpool.tile([128, 2 * TC], bf16)
            decT_sb = cpool.tile([128, 2 * U], bf16)
            peb_sb = cpool.tile([128, 4 * TC], f32)  # [ht*TC+t] = enc@We
            pd_sb = cpool.tile([128, 4 * U], f32)  # [ht*U+u] = dec@Wd + b1

            # Issue order = consumption order; wfc (needed ~7us in) last.
            nc.sync.dma_start(w1we_sb, w1we_d[:, :])
            nc.sync.dma_start(encT_sb, encT_d[:, :])
            nc.sync.dma_start(w1wd_sb, w1wd_d[:, :])
            nc.sync.dma_start(decT_sb, decT_d[:, :])
            nc.sync.dma_start(b1_sb, b1_d[:, :])
            nc.sync.dma_start(wfc_sb, wfc_d[:, :])

            # Prologue: peb[h,t] = enc@We ; pd[h,u] = dec@Wd + b1.
            # PSUM evacuation on DVE (idle early) so ACT can start the first
            # groups' bias-fused gelus as soon as each (peb, pd) block lands.
            for ht in range(4):
                pe_ps = pro_ps.tile([128, TC], f32)
                for di in range(2):
                    nc.tensor.matmul(
                        pe_ps,
                        w1we_sb[:, di * H + ht * 128 : di * H + (ht + 1) * 128],
                        encT_sb[:, di * TC : (di + 1) * TC],
                        start=(di == 0),
                        stop=(di == 1),
                    )
                nc.vector.tensor_copy(peb_sb[:, ht * TC : (ht + 1) * TC], pe_ps)
                pd_ps = pro_ps.tile([128, U], f32)
                for di in range(2):
                    nc.tensor.matmul(
                        pd_ps,
                        w1wd_sb[:, di * H + ht * 128 : di * H + (ht + 1) * 128],
                        decT_sb[:, di * U : (di + 1) * U],
                        start=(di == 0),
                        stop=(di == 1),
                    )
                nc.vector.tensor_scalar_add(
                    pd_sb[:, ht * U : (ht + 1) * U],
                    pd_ps,
                    b1_sb[:, ht : ht + 1],
                )

            # Broadcast-add source APs for h-blocks 1..3, iteration order
            # (u, ht, t): peb u-dim stride 0; pd t-dim stride 0.
            peb_bc = (
                peb_sb[:, TC : 4 * TC]
                .rearrange("p (i t) -> p i t", i=3)
                .unsqueeze(1)
                .broadcast_to((128, UB, 3, TC))
            )
            pd_iu = pd_sb.rearrange("p (i u) -> p i u", i=4)

            # Main loop over groups of UB u's.
            for g in range(NG):
                u0 = g * UB
                hact = hpool.tile([128, UB * 4 * TC], bf16, tag="hact")
                if g < 3:
                    # First groups: all h-blocks via ACT bias-fused gelu —
                    # no GPSIMD add in the dependence chain, so PE's output
                    # matmuls start while the prologue is still draining.
                    for ui in range(UB):
                        for ht in range(4):
                            nc.scalar.activation(
                                hact[
                                    :, ui * 4 * TC + ht * TC : ui * 4 * TC + (ht + 1) * TC
                                ],
                                peb_sb[:, ht * TC : (ht + 1) * TC],
                                GELU,
                                bias=pd_sb[:, ht * U + u0 + ui : ht * U + u0 + ui + 1],
                            )
                else:
                    # h-block 0: gelu straight from peb with pd as
                    # per-partition bias — skips the explicit add.
                    for ui in range(UB):
                        nc.scalar.activation(
                            hact[:, ui * 4 * TC : ui * 4 * TC + TC],
                            peb_sb[:, 0:TC],
                            GELU,
                            bias=pd_sb[:, u0 + ui : u0 + ui + 1],
                        )
                    # h-blocks 1..3: GPSIMD broadcast add, then one big gelu.
                    tmp = tpool.tile([128, UB * 3 * TC], f32, tag="tmp")
                    pd_bc = (
                        pd_iu[:, 1:4, u0 : u0 + UB]
                        .transpose([0, 2, 1])
                        .unsqueeze(3)
                        .broadcast_to((128, UB, 3, TC))
                    )
                    nc.gpsimd.tensor_tensor(
                        tmp.rearrange("p (u i t) -> p u i t", u=UB, i=3),
                        peb_bc,
                        pd_bc,
                        mybir.AluOpType.add,
                    )
                    nc.scalar.activation(
                        hact.rearrange("p (u x) -> p u x", u=UB)[:, :, TC : 4 * TC],
                        tmp.rearrange("p (u x) -> p u x", u=UB),
                        GELU,
                    )

                # psum tile (128 t, 2 banks): [:, ui*512:+512] = out rows for
                # (t-block ts, u0+ui); contraction over 4 h-blocks. The final
                # group drains its two halves on separate engines/queues so
                # the tail after the last matmul is ~halved.
                last = g == NG - 1
                for ts in range(TC // 128):
                    ops = out_ps_pool.tile([128, UB * V], f32)
                    for ui in range(UB):
                        for ht in range(4):
                            nc.tensor.matmul(
                                ops[:, ui * V : (ui + 1) * V],
                                hact[
                                    :,
                                    ui * 4 * TC
                                    + ht * TC
                                    + ts * 128 : ui * 4 * TC
                                    + ht * TC
                                    + ts * 128
                                    + 128,
                                ],
                                wfc_sb[:, ht * V : (ht + 1) * V],
                                start=(ht == 0),
                                stop=(ht == 3),
                            )
                    osb = osb_pool.tile([128, UB * V], bf16)
                    if last and ts == 1:
                        nc.scalar.copy(osb, ops)
                        nc.scalar.dma_start(
                            out_d[ts * 128 : (ts + 1) * 128, u0 : u0 + UB, :],
                            osb.rearrange("p (u v) -> p u v", u=UB),
                        )
                    else:
                        nc.vector.tensor_copy(osb, ops)
                        nc.sync.dma_start(
                            out_d[ts * 128 : (ts + 1) * 128, u0 : u0 + UB, :],
                            osb.rearrange("p (u v) -> p u v", u=UB),
                        )

    nc.compile()
    _PROGRAM = nc
    return nc


def kernel(enc, dec, W1, b1, Wfc):
    global LAST_RESULT
    nc = _build()
    bf = ml_dtypes.bfloat16
    enc = np.asarray(enc, dtype=np.float32)
    dec = np.asarray(dec, dtype=np.float32)
    W1 = np.asarray(W1, dtype=np.float32)
    b1 = np.asarray(b1, dtype=np.float32)
    Wfc = np.asarray(Wfc, dtype=np.float32)

    # Pre-tile to partition-major (128, free) SBUF layouts.
    def pmaj(x, nblk):  # (nblk*128, F) -> (128, nblk*F)
        F = x.shape[1]
        return np.ascontiguousarray(
            x.reshape(nblk, 128, F).transpose(1, 0, 2).reshape(128, nblk * F)
        )

    w1we = pmaj(W1[:D], 2).astype(bf)
    w1wd = pmaj(W1[D:], 2).astype(bf)
    wfct = pmaj(Wfc, 4).astype(bf)
    b1t = np.ascontiguousarray(b1.reshape(4, 128).T)

    in_maps = []
    for c in range(NCORES):
        b, t0 = c // 2, (c % 2) * TC
        in_maps.append(
            {
                "encT": pmaj(enc[b, t0 : t0 + TC, :].T, 2).astype(bf),
                "decT": pmaj(dec[b].T, 2).astype(bf),
                "w1we": w1we,
                "w1wd": w1wd,
                "b1": b1t,
                "Wfc": wfct,
            }
        )

    LAST_RESULT = run_bass_kernel_spmd(nc, in_maps, list(range(NCORES)))

    out = np.empty((B, T, U, V), np.float32)
    for c in range(NCORES):
        b, t0 = c // 2, (c % 2) * TC
        out[b, t0 : t0 + TC] = LAST_RESULT.results[c]["out"].astype(np.float32)
    return out
